# revision 2
# baseline (speedup 1.0000x reference)
"""Causal self-attention on 8 trn2 NeuronCores.

Sharding: data-parallel over batch (2) x tensor-parallel over heads (4/core).
Core c handles batch c//4, heads (c%4)*4 .. (c%4)*4+4.  Each core computes
QKV projection for its heads, causal attention, and a partial c_proj
(y_local @ w_proj[local rows]); the host sums the 4 partials per batch
(b_proj is folded in as b_proj/4 on every core).

Device kernel notes (v2):
- Matmul inputs are bf16 (host-converted); accumulation stays fp32 in PSUM.
- Attention uses the S^T = K Q^T orientation so the softmax reduction is a
  matmul: V is augmented with a ones column, so A@V also yields the softmax
  denominator; normalization is a per-query free-dim scale applied after a
  gpsimd partition_broadcast of reciprocal_approx_fast(denominator).
- exp runs without max-subtraction (scores are bounded ~|3| for this
  problem family; a masked entry's exp(-1e9) underflow to 0 matches the
  reference's softmax exactly).  Causality is tile-skipped; diagonal band
  tiles get one in-place multiply with a static 128x128 triu mask.
- Scores for 2 kt tiles x 2 heads are staged in a 4-bank PSUM tile so one
  ACTIVATE covers [128, 2048]: 40 exp calls instead of 160 (the ~350-cycle
  per-call ACT overhead and per-call semaphores dominated the scalar queue).
- Inputs arrive in 10 large DMA descriptors (vs ~46): descriptor issue on
  the sync queue costs ~600ns each and serialized the first 34us.  x^T
  loads in 4 column-block descriptors (tokens 512g..512g+511) so query
  group g's projections start as soon as its token block + weights land.
- c_proj of each query group runs right after its attention, spreading the
  output DMA across the main loop instead of a serial tail.
"""

import os
import sys

for p in ("/root/.axon_site", "/root/.axon_site/_ro/trn_rl_repo", "/root/.axon_site/_ro/pypackages", "/opt/trn_rl_repo"):
    if os.path.isdir(p) and p not in sys.path:
        sys.path.append(p)

import ml_dtypes
import numpy as np

import concourse.bacc as bacc
import concourse.mybir as mybir
import concourse.tile as tile
from concourse.bass_utils import run_bass_kernel_spmd

F32 = mybir.dt.float32
BF16 = mybir.dt.bfloat16
Exp = mybir.ActivationFunctionType.Exp
MULT = mybir.AluOpType.mult
ADD = mybir.AluOpType.add
BF = ml_dtypes.bfloat16

T = 2048            # sequence length (per batch)
C = 1024            # embedding dim
NHL = 4             # heads per core
HD = 64             # head dim
FL = NHL * HD       # local features (256)
CK = C // 128       # contraction chunks (8)
W3 = 3 * FL         # packed weight row: [wk | wq | wv] (768)
NQG = T // 512      # query groups of 512 (4)
NTT = T // 128      # token tiles of 128 (16)

_CACHE = {}
LAST_RESULTS = None


def _build():
    nc = bacc.Bacc("TRN2", target_bir_lowering=False, debug=False, num_devices=8)

    xT = nc.dram_tensor("xT", [C, T], BF16, kind="ExternalInput").ap()
    wkqv = nc.dram_tensor("wkqv", [C, W3], BF16, kind="ExternalInput").ap()
    wp = nc.dram_tensor("wp", [FL, C], BF16, kind="ExternalInput").ap()
    bkq = nc.dram_tensor("bkq", [128, 4], F32, kind="ExternalInput").ap()
    bv = nc.dram_tensor("bv", [1, FL], F32, kind="ExternalInput").ap()
    bpf = nc.dram_tensor("bpf", [1, C], F32, kind="ExternalInput").ap()
    tril = nc.dram_tensor("tril", [128, 128], BF16, kind="ExternalInput").ap()
    vones = nc.dram_tensor("vones", [128, NTT * NHL], BF16, kind="ExternalInput").ap()
    out = nc.dram_tensor("out", [T, C], F32, kind="ExternalOutput").ap()

    with tile.TileContext(nc) as tc:
        with (
            tc.tile_pool(name="persist", bufs=1) as pp,
            tc.tile_pool(name="xpool", bufs=1) as xp,
            tc.tile_pool(name="attp", bufs=3) as ap_,
            tc.tile_pool(name="smallp", bufs=2) as sp,
            tc.tile_pool(name="outp", bufs=2) as op_,
            tc.tile_pool(name="proj_ps", bufs=2, space="PSUM") as pps,
            tc.tile_pool(name="stg_ps", bufs=1, space="PSUM") as sps,
            tc.tile_pool(name="av_ps", bufs=1, space="PSUM") as avps,
        ):
            # ---- persistent SBUF tensors ----
            xt_sb = xp.tile([128, CK * T], BF16)          # 8 chunks of x^T [128, 2048]
            w_sb = pp.tile([128, CK * W3], BF16)          # 8 chunks of [wk|wq|wv]
            qt_sb = [pp.tile([128, T], BF16, tag=f"qt{p}", name=f"qt{p}") for p in range(2)]
            kt_sb = [pp.tile([128, T], BF16, tag=f"kt{p}", name=f"kt{p}") for p in range(2)]
            v_sb = pp.tile([128, NTT * NHL * (HD + 1)], BF16, tag="v")  # per tile: 4x65
            yt_sb = [pp.tile([128, T], BF16, tag=f"yt{p}", name=f"yt{p}") for p in range(2)]
            wp_sb = pp.tile([128, 2 * C], BF16)
            bkq_sb = pp.tile([128, 4], F32, tag="bkq")
            bv_bc = pp.tile([128, FL], F32, tag="bvbc")
            bp_bc = pp.tile([128, C], F32, tag="bpbc")
            bp_row = pp.tile([1, C], F32, tag="bprowf")
            tril_sb = pp.tile([128, 128], BF16, tag="tril")
            bv_row = pp.tile([1, FL], F32, tag="bvrow")
            vones_sb = pp.tile([128, NTT * NHL], BF16, tag="vones")

            # ---- input DMAs: few large descriptors, split across both
            # hardware DGE queues (sync + scalar) so issue is not serial ----
            xt_v = xt_sb[:].rearrange("p (ck t) -> p ck t", ck=CK)
            xT_v = xT.rearrange("(ck p) t -> p ck t", p=128)
            for g in range(NQG):
                nc.sync.dma_start(out=xt_v[:, :, g * 512:(g + 1) * 512],
                                  in_=xT_v[:, :, g * 512:(g + 1) * 512])
            nc.scalar.dma_start(out=w_sb[:].rearrange("p (ck f) -> p ck f", ck=CK),
                                in_=wkqv.rearrange("(ck p) f -> p ck f", p=128))
            nc.scalar.dma_start(out=wp_sb[:].rearrange("p (f c) -> p f c", f=2),
                                in_=wp.rearrange("(f p) c -> p f c", p=128))
            nc.scalar.dma_start(out=bkq_sb[:], in_=bkq)
            nc.scalar.dma_start(out=tril_sb[:], in_=tril)
            nc.scalar.dma_start(out=bv_row[:], in_=bv)
            nc.scalar.dma_start(out=vones_sb[:], in_=vones)
            nc.scalar.dma_start(out=bp_row[:], in_=bpf)
            nc.gpsimd.partition_broadcast(bv_bc[:], bv_row[:])
            nc.gpsimd.partition_broadcast(bp_bc[:], bp_row[:])
            # ones column of the augmented V (col 64 of each head block),
            # written by DVE so it serializes with the DVE value writes
            v_ones = v_sb[:].rearrange("p (n c) -> p n c", c=HD + 1)[:, :, HD]
            nc.vector.tensor_copy(v_ones, vones_sb[:])

            def v_tile(tt):
                return v_sb[:, tt * NHL * (HD + 1):(tt + 1) * NHL * (HD + 1)]

            # ---- per-query-group pipeline ----
            for qg in range(NQG):
                qs = qg * 512
                # K^T and Q^T for this query group, both head pairs
                for w_off, bcol, dst in ((0, 0, kt_sb), (FL, 2, qt_sb)):
                    for p in range(2):
                        ps = pps.tile([128, 512], F32, tag="proj", name="proj_ps")
                        for ck in range(CK):
                            nc.tensor.matmul(
                                ps[:],
                                w_sb[:, ck * W3 + w_off + p * 128: ck * W3 + w_off + (p + 1) * 128],
                                xt_sb[:, ck * T + qs: ck * T + qs + 512],
                                start=(ck == 0), stop=(ck == CK - 1))
                        nc.vector.tensor_scalar_add(dst[p][:, qs:qs + 512], ps[:],
                                                    bkq_sb[:, bcol + p:bcol + p + 1])
                # V for this group's token tiles (natural orientation + bias)
                for tt in range(4 * qg, 4 * qg + 4):
                    ps = pps.tile([128, 512], F32, tag="proj", name="proj_ps")
                    for ck in range(CK):
                        nc.tensor.matmul(
                            ps[:, 0:FL],
                            xt_sb[:, ck * T + tt * 128: ck * T + (tt + 1) * 128],
                            w_sb[:, ck * W3 + 2 * FL:(ck + 1) * W3],
                            start=(ck == 0), stop=(ck == CK - 1))
                    vdst = v_tile(tt).rearrange("p (n c) -> p n c", c=HD + 1)[:, :, 0:HD]
                    nc.vector.tensor_tensor(vdst, ps[:, 0:FL].rearrange("p (n c) -> p n c", c=HD),
                                            bv_bc[:].rearrange("p (n c) -> p n c", c=HD), ADD)

                # attention for this query group, per head pair
                for p in range(2):
                    # av: h0 in bank cols 0:512, h1 in 512:1024; row 64 = denominator
                    av = avps.tile([65, 1024], F32, tag="av", name="av_ps")
                    ngrp = 2 * qg + 2
                    for g in range(ngrp):
                        stg = sps.tile([128, 2048], F32, tag="stg", name="stg_ps")
                        att = ap_.tile([128, 2048], BF16, tag="att", name="att")
                        for ktl in range(2):
                            kt = 2 * g + ktl
                            d = max(0, (kt - 4 * qg) * 128)
                            for h in range(2):
                                s = 2 * ktl + h
                                nc.tensor.matmul(
                                    stg[:, s * 512 + d:(s + 1) * 512],
                                    kt_sb[p][h * 64:(h + 1) * 64, kt * 128:(kt + 1) * 128],
                                    qt_sb[p][h * 64:(h + 1) * 64, qs + d: qs + 512],
                                    start=True, stop=True)
                        # one exp over all 4 segments (stale-psum columns below the
                        # causal offset d produce garbage that is never read)
                        nc.scalar.activation(att[:], stg[:], Exp, scale=0.125)
                        for ktl in range(2):
                            kt = 2 * g + ktl
                            if kt >= 4 * qg:
                                d = (kt - 4 * qg) * 128
                                for h in range(2):
                                    s = 2 * ktl + h
                                    a = att[:, s * 512 + d: s * 512 + d + 128]
                                    nc.vector.tensor_tensor(a, a, tril_sb[:], MULT)
                        for ktl in range(2):
                            kt = 2 * g + ktl
                            d = max(0, (kt - 4 * qg) * 128)
                            for h in range(2):
                                s = 2 * ktl + h
                                nc.tensor.matmul(
                                    av[:, h * 512 + d:(h + 1) * 512],
                                    v_tile(kt)[:, (2 * p + h) * (HD + 1):(2 * p + h + 1) * (HD + 1)],
                                    att[:, s * 512 + d:(s + 1) * 512],
                                    start=(kt == 0), stop=(kt == 4 * qg + 3))
                    # normalization: one batched chain for both heads
                    dsb = sp.tile([1, 1024], F32, tag="dsb", name="dsb")
                    dinv = sp.tile([1, 1024], F32, tag="dinv", name="dinv")
                    bc = sp.tile([64, 1024], F32, tag="bc", name="bc")
                    # custom-DVE reciprocal misreads PSUM at partition offset 64
                    # on HW; stage the row through SBUF
                    nc.vector.tensor_copy(dsb[:], av[64:65, :])
                    nc.vector.reciprocal_approx_fast(out=dinv[:], in_=dsb[:])
                    nc.gpsimd.partition_broadcast(bc[:], dinv[:])
                    for h in range(2):
                        nc.vector.tensor_tensor(yt_sb[p][h * 64:(h + 1) * 64, qs:qs + 512],
                                                av[0:64, h * 512:(h + 1) * 512],
                                                bc[:, h * 512:(h + 1) * 512], MULT)

                # c_proj partial for this query group's token tiles
                for tt in range(4 * qg, 4 * qg + 4):
                    ob = op_.tile([128, C], F32, tag="ob", name="ob")
                    for ng in range(2):
                        ps = pps.tile([128, 512], F32, tag="proj", name="proj_ps")
                        for f in range(2):
                            nc.tensor.matmul(
                                ps[:],
                                yt_sb[f][:, tt * 128:(tt + 1) * 128],
                                wp_sb[:, f * C + ng * 512: f * C + ng * 512 + 512],
                                start=(f == 0), stop=(f == 1))
                        nc.vector.tensor_tensor(ob[:, ng * 512:(ng + 1) * 512], ps[:],
                                                bp_bc[:, ng * 512:(ng + 1) * 512], ADD)
                    nc.sync.dma_start(out=out[tt * 128:(tt + 1) * 128, :], in_=ob[:])

    nc.compile()
    return nc


def kernel(x, w_attn, b_attn, w_proj, b_proj):
    global LAST_RESULTS
    x = np.asarray(x, dtype=np.float32)
    w_attn = np.asarray(w_attn, dtype=np.float32)
    b_attn = np.asarray(b_attn, dtype=np.float32)
    w_proj = np.asarray(w_proj, dtype=np.float32)
    b_proj = np.asarray(b_proj, dtype=np.float32)
    b, t, c = x.shape
    assert (b, t, c) == (2, T, C)

    if "nc" not in _CACHE:
        _CACHE["nc"] = _build()
    nc = _CACHE["nc"]

    trilm = np.triu(np.ones((128, 128), dtype=np.float32))  # [k, q]: valid iff k <= q
    in_maps = []
    for core in range(8):
        bi, g = divmod(core, 4)
        cs = FL * g  # column/row offset for this core's 4 heads
        wk = w_attn[:, C + cs:C + cs + FL]
        wq = w_attn[:, cs:cs + FL]
        wv = w_attn[:, 2 * C + cs:2 * C + cs + FL]
        bk = b_attn[C + cs:C + cs + FL]
        bq = b_attn[cs:cs + FL]
        bkq = np.stack([bk[0:128], bk[128:256], bq[0:128], bq[128:256]], axis=1)
        in_maps.append({
            "xT": np.ascontiguousarray(x[bi].T).astype(BF),
            "wkqv": np.ascontiguousarray(np.concatenate([wk, wq, wv], axis=1)).astype(BF),
            "wp": np.ascontiguousarray(w_proj[cs:cs + FL, :]).astype(BF),
            "bkq": np.ascontiguousarray(bkq),
            "bv": np.ascontiguousarray(b_attn[2 * C + cs:2 * C + cs + FL].reshape(1, FL)),
            "bpf": (b_proj / 4.0).reshape(1, C),
            "tril": trilm.astype(BF),
            "vones": np.ones((128, NTT * NHL), dtype=BF),
        })

    res = run_bass_kernel_spmd(nc, in_maps, core_ids=list(range(8)))
    LAST_RESULTS = res
    # unshard: sum the 4 tensor-parallel partials of each batch element
    y = np.empty((2, T, C), dtype=np.float32)
    for bi in range(2):
        acc = res.results[4 * bi]["out"].astype(np.float32)
        for g in range(1, 4):
            acc = acc + res.results[4 * bi + g]["out"]
        y[bi] = acc
    return y


# revision 5
# speedup vs baseline: 1.6672x; 1.6672x over previous
"""Causal self-attention on 8 trn2 NeuronCores.

Sharding: data-parallel over batch (2) x tensor-parallel over heads (4/core).
Core c handles batch c//4, heads (c%4)*4 .. (c%4)*4+4.  Each core computes
QKV projection for its heads, causal attention, and a partial c_proj
(y_local @ w_proj[local rows]); the host sums the 4 partials per batch
(b_proj is folded in as b_proj/4 on every core).

Device kernel notes (v3):
- Matmul inputs are bf16 (host-converted); accumulation stays fp32 in PSUM.
- Attention uses the S^T = K Q^T orientation so the softmax reduction is a
  matmul: V is augmented with a ones column (col 64), so A@V also yields the
  softmax denominator in psum row 64.
- exp runs without max-subtraction (scores bounded for this problem family).
- Per kt tile, both heads' scores land in one 2-bank PSUM tile [128,1024]
  so one ACTIVATE covers both heads: 80 exp calls instead of 160 (per-call
  ACT overhead + per-call semaphores dominated the scalar queue at 160).
- The per-(qg,p) kt loop is software-pipelined with lag 2 (emit scores(kt),
  exp(kt), then AV(kt-2)): engine queues are strict FIFO, so without the
  lag the PE queue head blocks on the exp of the same tile and the PE goes
  idle (and HAM-cold) for ~2us per tile.
- AV psum is evacuated to yt (unnormalized) immediately so the next head
  pair can reuse the av psum tile; the 1/denominator scale is applied
  in-place on yt off the critical path.  c_proj of query group qg is
  emitted after the projections of qg+1 for the same reason.
- Inputs arrive in 10 large DMA descriptors split over both HWDGE queues
  (sync + scalar); descriptor issue costs ~600ns each and serialized the
  first 34us when done one tile at a time.  x^T loads in 4 column-block
  descriptors so query group g's projections start once its block lands.
"""

import os
import sys

for p in ("/root/.axon_site", "/root/.axon_site/_ro/trn_rl_repo", "/root/.axon_site/_ro/pypackages", "/opt/trn_rl_repo"):
    if os.path.isdir(p) and p not in sys.path:
        sys.path.append(p)

import ml_dtypes
import numpy as np

import concourse.bacc as bacc
import concourse.mybir as mybir
import concourse.tile as tile
from concourse.bass_utils import run_bass_kernel_spmd

F32 = mybir.dt.float32
BF16 = mybir.dt.bfloat16
Exp = mybir.ActivationFunctionType.Exp
MULT = mybir.AluOpType.mult
ADD = mybir.AluOpType.add
BF = ml_dtypes.bfloat16

T = 2048            # sequence length (per batch)
C = 1024            # embedding dim
NHL = 4             # heads per core
HD = 64             # head dim
FL = NHL * HD       # local features (256)
CK = C // 128       # contraction chunks (8)
W3 = 3 * FL         # packed weight row: [wk | wq | wv] (768)
NQG = T // 512      # query groups of 512 (4)
NTT = T // 128      # token tiles of 128 (16)

_CACHE = {}
LAST_RESULTS = None


def _build():
    nc = bacc.Bacc("TRN2", target_bir_lowering=False, debug=False, num_devices=8)

    xT = nc.dram_tensor("xT", [C, T], BF16, kind="ExternalInput").ap()
    wkqv = nc.dram_tensor("wkqv", [C, W3], BF16, kind="ExternalInput").ap()
    wp = nc.dram_tensor("wp", [FL, C], BF16, kind="ExternalInput").ap()
    bkq = nc.dram_tensor("bkq", [128, 4], F32, kind="ExternalInput").ap()
    bv = nc.dram_tensor("bv", [1, FL], F32, kind="ExternalInput").ap()
    bpf = nc.dram_tensor("bpf", [1, C], F32, kind="ExternalInput").ap()
    tril2 = nc.dram_tensor("tril2", [128, 256], BF16, kind="ExternalInput").ap()
    vones = nc.dram_tensor("vones", [128, NTT * NHL], BF16, kind="ExternalInput").ap()
    out = nc.dram_tensor("out", [T, C], F32, kind="ExternalOutput").ap()

    with tile.TileContext(nc) as tc:
        with (
            tc.tile_pool(name="persist", bufs=1) as pp,
            tc.tile_pool(name="xpool", bufs=1) as xp,
            tc.tile_pool(name="attp", bufs=4) as ap_,
            tc.tile_pool(name="smallp", bufs=2) as sp,
            tc.tile_pool(name="outp", bufs=2) as op_,
            tc.tile_pool(name="proj_ps", bufs=2, space="PSUM") as pps,
            tc.tile_pool(name="stg_ps", bufs=2, space="PSUM") as sps,
            tc.tile_pool(name="av_ps", bufs=1, space="PSUM") as avps,
        ):
            # ---- persistent SBUF tensors ----
            xt_sb = xp.tile([128, CK * T], BF16)          # 8 chunks of x^T [128, 2048]
            w_sb = pp.tile([128, CK * W3], BF16)          # 8 chunks of [wk|wq|wv]
            qt_sb = [pp.tile([128, T], BF16, tag=f"qt{p}", name=f"qt{p}") for p in range(2)]
            kt_sb = [pp.tile([128, T], BF16, tag=f"kt{p}", name=f"kt{p}") for p in range(2)]
            v_sb = pp.tile([128, NTT * NHL * (HD + 1)], BF16, tag="v")  # per tile: 4x65
            yt_sb = [pp.tile([128, T], BF16, tag=f"yt{p}", name=f"yt{p}") for p in range(2)]
            wp_sb = pp.tile([128, 2 * C], BF16)
            bkq_sb = pp.tile([128, 4], F32, tag="bkq")
            bv_bc = pp.tile([128, FL], F32, tag="bvbc")
            bp_bc = pp.tile([128, C], F32, tag="bpbc")
            bp_row = pp.tile([1, C], F32, tag="bprowf")
            tril_sb = pp.tile([128, 256], BF16, tag="tril")
            bv_row = pp.tile([1, FL], F32, tag="bvrow")
            vones_sb = pp.tile([128, NTT * NHL], BF16, tag="vones")

            # ---- input DMAs: few large descriptors on both HWDGE queues ----
            xt_v = xt_sb[:].rearrange("p (ck t) -> p ck t", ck=CK)
            xT_v = xT.rearrange("(ck p) t -> p ck t", p=128)
            for g in range(NQG):
                nc.sync.dma_start(out=xt_v[:, :, g * 512:(g + 1) * 512],
                                  in_=xT_v[:, :, g * 512:(g + 1) * 512])
            nc.scalar.dma_start(out=w_sb[:].rearrange("p (ck f) -> p ck f", ck=CK),
                                in_=wkqv.rearrange("(ck p) f -> p ck f", p=128))
            nc.scalar.dma_start(out=wp_sb[:].rearrange("p (f c) -> p f c", f=2),
                                in_=wp.rearrange("(f p) c -> p f c", p=128))
            nc.scalar.dma_start(out=bkq_sb[:], in_=bkq)
            nc.scalar.dma_start(out=tril_sb[:], in_=tril2)
            nc.scalar.dma_start(out=bv_row[:], in_=bv)
            nc.scalar.dma_start(out=vones_sb[:], in_=vones)
            nc.scalar.dma_start(out=bp_row[:], in_=bpf)
            nc.gpsimd.partition_broadcast(bv_bc[:], bv_row[:])
            nc.gpsimd.partition_broadcast(bp_bc[:], bp_row[:])
            # ones column of the augmented V (col 64 of each head block)
            v_ones = v_sb[:].rearrange("p (n c) -> p n c", c=HD + 1)[:, :, HD]
            nc.vector.tensor_copy(v_ones, vones_sb[:])

            def v_tile(tt):
                return v_sb[:, tt * NHL * (HD + 1):(tt + 1) * NHL * (HD + 1)]

            def emit_proj(qg):
                """K^T, Q^T, V projections for query group qg."""
                qs = qg * 512
                for w_off, bcol, dst in ((0, 0, kt_sb), (FL, 2, qt_sb)):
                    for p in range(2):
                        ps = pps.tile([128, 512], F32, tag="proj", name="proj_ps")
                        for ck in range(CK):
                            nc.tensor.matmul(
                                ps[:],
                                w_sb[:, ck * W3 + w_off + p * 128: ck * W3 + w_off + (p + 1) * 128],
                                xt_sb[:, ck * T + qs: ck * T + qs + 512],
                                start=(ck == 0), stop=(ck == CK - 1))
                        nc.vector.tensor_scalar_add(dst[p][:, qs:qs + 512], ps[:],
                                                    bkq_sb[:, bcol + p:bcol + p + 1])
                for tt in range(4 * qg, 4 * qg + 4):
                    ps = pps.tile([128, 512], F32, tag="proj", name="proj_ps")
                    for ck in range(CK):
                        nc.tensor.matmul(
                            ps[:, 0:FL],
                            xt_sb[:, ck * T + tt * 128: ck * T + (tt + 1) * 128],
                            w_sb[:, ck * W3 + 2 * FL:(ck + 1) * W3],
                            start=(ck == 0), stop=(ck == CK - 1))
                    vdst = v_tile(tt).rearrange("p (n c) -> p n c", c=HD + 1)[:, :, 0:HD]
                    nc.vector.tensor_tensor(vdst, ps[:, 0:FL].rearrange("p (n c) -> p n c", c=HD),
                                            bv_bc[:].rearrange("p (n c) -> p n c", c=HD), ADD)

            def emit_attention(qg, p):
                """Causal attention for query group qg, head pair p (lag-2 pipeline)."""
                qs = qg * 512
                K = 4 * qg + 4
                atts = [None] * K

                def emit_scores(kt):
                    d = max(0, (kt - 4 * qg) * 128)
                    stg = sps.tile([128, 1024], F32, tag="stg", name="stg_ps")
                    att = ap_.tile([128, 1024], BF16, tag="att", name="att")
                    atts[kt] = att
                    for h in range(2):
                        nc.tensor.matmul(
                            stg[:, h * 512 + d:(h + 1) * 512],
                            kt_sb[p][h * 64:(h + 1) * 64, kt * 128:(kt + 1) * 128],
                            qt_sb[p][h * 64:(h + 1) * 64, qs + d: qs + 512],
                            start=True, stop=True)
                    # one exp for both heads (stale-psum columns below the causal
                    # offset d produce garbage that is never read)
                    nc.scalar.activation(att[:], stg[:], Exp, scale=0.125)
                    if kt >= 4 * qg:
                        # in-place causal mask, both heads in one strided op
                        a = att[:].rearrange("q (h j) -> q h j", h=2)[:, :, d:d + 128]
                        m = tril_sb[:].rearrange("q (h j) -> q h j", h=2)
                        nc.vector.tensor_tensor(a, a, m, MULT)

                def emit_av(kt):
                    d = max(0, (kt - 4 * qg) * 128)
                    for h in range(2):
                        nc.tensor.matmul(
                            av[:, h * 512 + d:(h + 1) * 512],
                            v_tile(kt)[:, (2 * p + h) * (HD + 1):(2 * p + h + 1) * (HD + 1)],
                            atts[kt][:, h * 512 + d:(h + 1) * 512],
                            start=(kt == 0), stop=(kt == K - 1))

                # av: h0 in bank cols 0:512, h1 in 512:1024; row 64 = denominator
                av = avps.tile([65, 1024], F32, tag="av", name="av_ps")
                for kt in range(K):
                    emit_scores(kt)
                    if kt >= 2:
                        emit_av(kt - 2)
                emit_av(K - 2)
                emit_av(K - 1)
                # evacuate av quickly (unnormalized), then scale in-place later
                dsb = sp.tile([1, 1024], F32, tag="dsb", name="dsb")
                dinv = sp.tile([1, 1024], F32, tag="dinv", name="dinv")
                # 128 partitions so the h=1 in-place scale below can read a
                # base-partition-64 slice (DVE SB+SB ops need equal bases)
                bc = sp.tile([128, 1024], F32, tag="bc", name="bc")
                # custom-DVE reciprocal misreads PSUM at partition offset 64 on
                # HW; stage the denominator row through SBUF
                nc.vector.tensor_copy(dsb[:], av[64:65, :])
                for h in range(2):
                    nc.vector.tensor_copy(yt_sb[p][h * 64:(h + 1) * 64, qs:qs + 512],
                                          av[0:64, h * 512:(h + 1) * 512])
                nc.vector.reciprocal_approx_fast(out=dinv[:], in_=dsb[:])
                nc.gpsimd.partition_broadcast(bc[:], dinv[:])
                for h in range(2):
                    y = yt_sb[p][h * 64:(h + 1) * 64, qs:qs + 512]
                    nc.vector.tensor_tensor(
                        y, y, bc[h * 64:(h + 1) * 64, h * 512:(h + 1) * 512], MULT)

            def emit_cproj(qg):
                """c_proj partial + output DMA for query group qg's token tiles."""
                for tt in range(4 * qg, 4 * qg + 4):
                    ob = op_.tile([128, C], F32, tag="ob", name="ob")
                    for ng in range(2):
                        ps = pps.tile([128, 512], F32, tag="proj", name="proj_ps")
                        for f in range(2):
                            nc.tensor.matmul(
                                ps[:],
                                yt_sb[f][:, tt * 128:(tt + 1) * 128],
                                wp_sb[:, f * C + ng * 512: f * C + ng * 512 + 512],
                                start=(f == 0), stop=(f == 1))
                        nc.vector.tensor_tensor(ob[:, ng * 512:(ng + 1) * 512], ps[:],
                                                bp_bc[:, ng * 512:(ng + 1) * 512], ADD)
                    nc.sync.dma_start(out=out[tt * 128:(tt + 1) * 128, :], in_=ob[:])

            emit_proj(0)
            for qg in range(NQG):
                for p in range(2):
                    emit_attention(qg, p)
                if qg + 1 < NQG:
                    emit_proj(qg + 1)
                emit_cproj(qg)

    nc.compile()
    return nc


def kernel(x, w_attn, b_attn, w_proj, b_proj):
    global LAST_RESULTS
    x = np.asarray(x, dtype=np.float32)
    w_attn = np.asarray(w_attn, dtype=np.float32)
    b_attn = np.asarray(b_attn, dtype=np.float32)
    w_proj = np.asarray(w_proj, dtype=np.float32)
    b_proj = np.asarray(b_proj, dtype=np.float32)
    b, t, c = x.shape
    assert (b, t, c) == (2, T, C)

    if "nc" not in _CACHE:
        _CACHE["nc"] = _build()
    nc = _CACHE["nc"]

    trilm = np.triu(np.ones((128, 128), dtype=np.float32))  # [k, q]: valid iff k <= q
    in_maps = []
    for core in range(8):
        bi, g = divmod(core, 4)
        cs = FL * g  # column/row offset for this core's 4 heads
        wk = w_attn[:, C + cs:C + cs + FL]
        wq = w_attn[:, cs:cs + FL]
        wv = w_attn[:, 2 * C + cs:2 * C + cs + FL]
        bk = b_attn[C + cs:C + cs + FL]
        bq = b_attn[cs:cs + FL]
        bkq = np.stack([bk[0:128], bk[128:256], bq[0:128], bq[128:256]], axis=1)
        in_maps.append({
            "xT": np.ascontiguousarray(x[bi].T).astype(BF),
            "wkqv": np.ascontiguousarray(np.concatenate([wk, wq, wv], axis=1)).astype(BF),
            "wp": np.ascontiguousarray(w_proj[cs:cs + FL, :]).astype(BF),
            "bkq": np.ascontiguousarray(bkq),
            "bv": np.ascontiguousarray(b_attn[2 * C + cs:2 * C + cs + FL].reshape(1, FL)),
            "bpf": (b_proj / 4.0).reshape(1, C),
            "tril2": np.tile(trilm, (1, 2)).astype(BF),
            "vones": np.ones((128, NTT * NHL), dtype=BF),
        })

    res = run_bass_kernel_spmd(nc, in_maps, core_ids=list(range(8)))
    LAST_RESULTS = res
    # unshard: sum the 4 tensor-parallel partials of each batch element
    y = np.empty((2, T, C), dtype=np.float32)
    for bi in range(2):
        acc = res.results[4 * bi]["out"].astype(np.float32)
        for g in range(1, 4):
            acc = acc + res.results[4 * bi + g]["out"]
        y[bi] = acc
    return y
